# revision 23
# baseline (speedup 1.0000x reference)
"""GAT message-passing kernel for Trainium2 (8 NeuronCores, Bass/Tile).

Strategy (edge/graph parallelism, per the sharding hint):
  - Host: shard dst nodes into 8 fixed ranges of LOCN=12544. Per core, pack
    dst nodes into supertiles of <=128 node slots; each supertile holds 16
    edge chunks of 128 slots: 4 chunks per src bank (src space split into 4
    banks of 25088 rows so gather indices fit int16 for the SWDGE dma_gather
    instruction). Gather groups of GS=2 supertiles make each per-bank gather
    exactly 1024 indices (the SWDGE descriptor-ring capacity).
  - Device phase 1 (replicated): T2[n] = [feat@fc_w | el | er] as bf16 rows
    of 256 elems (512B stride). Phase 1b computes a per-core er table
    (er_loc) for the LOCAL dst range from a host-sliced feat input, keeping
    the SPMD program uniform while er-gather indices stay dst-local.
  - Device phase 2, per group (32 chunks, 4096 edge slots): 4 batched
    dma_gathers (one per src bank, 1024 rows each, round-robin over 4 SWDGE
    queues) fetch [feat_src|el]; 1 small dma_gather fetches er per NODE SLOT
    (256 rows); er is expanded slot->edge with one-hot matmuls (Pc) into a
    single PSUM bank. Compute ex = exp(leaky_relu(el+er)); build one-hot
    edge->slot P_t; 16 accumulating bf16 matmuls per supertile
    P^T @ [feat*ex | ex] into a [128 slots, 132] PSUM tile; divide by the
    summed ex (segment softmax denominator) and stream out rows.
"""

import numpy as np

# ---------------- problem constants (hardcoded; kernel.py is self-contained) ---
N = 100000
F = 128           # input feature dim (= contraction dim)
H = 4             # heads
D = 32            # dim per head
HD = H * D        # 128
TCOLS = F + 2 * H  # 136 = feat_src(128) + el(4) + er(4)
TROW = 256        # bf16 elems per T2 row (512B)
ML = HD + H       # 132 = msg cols + ex cols
NEG = 0.2
NCORES = 8

# ---------------- device tiling parameters ------------------------------------
NPAD = 100352     # node rows padded (= 8*LOCN = 4*BANK)
LOCN = NPAD // 8  # dst nodes per core = 12544
BANK = NPAD // 4  # src rows per bank = 25088
NB = 4            # src banks
SN = 128          # node slots per supertile
CB = 4            # chunks per bank per supertile
CHE = 128         # edges per chunk
GS = 2            # supertiles per gather group
GCH = NB * GS * CB  # chunks per group = 32
GED = GCH * CHE   # edge slots per group = 4096
SEG_PAD = SN      # seg value for padding edge slots (no one-hot match)
IXC = NB * 64 + 16  # eidx cols per group: 4 banks x 64 + 16 er-slot cols

# phase-1 layout
WCH = 2048        # featT columns per DMA load (16 tiles)
G1 = 8            # node tiles per T2 write


def _wrap16(idx):
    """[n] int -> [128, n//16] int16 in the 16-wrapped, core-replicated layout."""
    n = idx.shape[0]
    w = idx.astype(np.int16).reshape(n // 16, 16).T
    return np.tile(w, (8, 1))


def _pack(src, dst):
    """Host-side index preprocessing.

    Returns (per-core dicts of eidx/segd/segT arrays, node_maps, ngroups).
    """
    import ml_dtypes

    src = np.asarray(src, np.int64)
    dst = np.asarray(dst, np.int64)
    order = np.argsort(dst, kind="stable")
    s_src = src[order]
    s_dst = dst[order]
    core_of = s_dst // LOCN
    cuts = np.searchsorted(core_of, np.arange(NCORES + 1))

    packs = []
    for k in range(NCORES):
        lo = k * LOCN
        e0, e1 = cuts[k], cuts[k + 1]
        ksrc = s_src[e0:e1]
        nloc = s_dst[e0:e1] - lo
        kbank = ksrc // BANK

        degb = np.zeros((LOCN, NB), np.int64)
        np.add.at(degb, (nloc, kbank), 1)

        # greedy supertile packing in dst order
        sup_id = np.zeros(LOCN, np.int64)
        node_slot = np.zeros(LOCN, np.int64)
        cur, cnt_n = 0, 0
        cnt_b = np.zeros(NB, np.int64)
        cap = CB * CHE
        for n in range(LOCN):
            dnb = degb[n]
            if cnt_n >= SN or np.any(cnt_b + dnb > cap):
                cur += 1
                cnt_n = 0
                cnt_b[:] = 0
            sup_id[n] = cur
            node_slot[n] = cnt_n
            cnt_n += 1
            cnt_b += dnb
        packs.append((ksrc, nloc, kbank, sup_id, node_slot, cur + 1))

    nsup_max = max(p[5] for p in packs)
    ngroups = (nsup_max + GS - 1) // GS
    nsup_pad = ngroups * GS
    nchunk = ngroups * GCH

    edatas = []
    node_maps = []
    for k in range(NCORES):
        ksrc, nloc, kbank, sup_id, node_slot, nsup = packs[k]
        esup = sup_id[nloc]
        eslot = node_slot[nloc]
        # position within (sup, bank) group, preserving dst order
        key = esup * NB + kbank
        o = np.argsort(key, kind="stable")
        sk = key[o]
        starts = np.searchsorted(sk, np.arange(nsup * NB))
        cnts = np.diff(np.append(starts, len(sk)))
        pos = np.empty(len(sk), np.int64)
        pos[o] = np.arange(len(sk)) - np.repeat(starts, cnts)
        cb = pos // CHE
        assert cb.max(initial=0) < CB
        p = pos % CHE
        gc = ((esup // GS) * GCH + kbank * (GS * CB)
              + (esup % GS) * CB + cb)

        srcloc = np.zeros((CHE, nchunk), np.int16)
        srcloc[p, gc] = (ksrc - kbank * BANK).astype(np.int16)
        # host-built one-hot matrices (edge->slot and slot->edge), bf16
        pth = np.zeros((CHE, nchunk * SN), ml_dtypes.bfloat16)
        pth[p, gc * SN + eslot] = 1
        pch = np.zeros((SN, nchunk * CHE), ml_dtypes.bfloat16)
        pch[eslot, gc * CHE + p] = 1

        # er-slot node ids (dst-local), one per supertile slot
        nid = np.zeros(nsup_pad * SN, np.int16)
        nid[sup_id * SN + node_slot] = np.arange(LOCN).astype(np.int16)

        # eidx per group: [bank0 | bank1 | bank2 | bank3 | er-slots]
        eidx = np.zeros((128, ngroups * IXC), np.int16)
        for g in range(ngroups):
            c0 = g * GCH
            x0 = g * IXC
            for b in range(NB):
                run = srcloc[:, c0 + b * GS * CB:c0 + (b + 1) * GS * CB]
                eidx[:, x0 + b * 64:x0 + (b + 1) * 64] = \
                    _wrap16(run.T.reshape(-1))
            eidx[:, x0 + NB * 64:x0 + IXC] = \
                _wrap16(nid[g * GS * SN:(g + 1) * GS * SN])

        nm = np.full(nsup_pad * SN, -1, np.int64)
        nm[sup_id * SN + node_slot] = np.arange(LOCN) + k * LOCN
        nm[nm >= N] = -1
        edatas.append({"eidx": eidx, "ptd": pth, "pcd": pch})
        node_maps.append(nm)
    return edatas, node_maps, ngroups


def _build(ngroups, npad=NPAD, wch=WCH, g1=G1):
    """Build the per-core Bass program (identical across cores)."""
    import concourse.bacc as bacc
    import concourse.tile as tile
    import concourse.mybir as mybir

    F32 = mybir.dt.float32
    BF16 = mybir.dt.bfloat16
    I32 = mybir.dt.int32
    I16 = mybir.dt.int16
    AOT = mybir.AluOpType
    ACT = mybir.ActivationFunctionType

    nchunk = ngroups * GCH

    nc = bacc.Bacc("TRN2", target_bir_lowering=False, debug=False,
                   num_swdge_queues=4)
    featT = nc.dram_tensor("featT", [F, npad], BF16, kind="ExternalInput")
    featL = nc.dram_tensor("featL", [F, LOCN], BF16, kind="ExternalInput")
    fcw = nc.dram_tensor("fcw", [F, HD], F32, kind="ExternalInput")
    attn = nc.dram_tensor("attn", [1, 2 * HD], F32, kind="ExternalInput")
    eidx = nc.dram_tensor("eidx", [128, ngroups * IXC], I16,
                          kind="ExternalInput")
    ptd = nc.dram_tensor("ptd", [CHE, nchunk * SN], BF16,
                         kind="ExternalInput")
    pcd = nc.dram_tensor("pcd", [SN, nchunk * CHE], BF16,
                         kind="ExternalInput")
    T2b = [nc.dram_tensor(f"T2b{b}", [BANK, TROW], BF16, kind="Internal")
           for b in range(NB)]
    erloc = nc.dram_tensor("erloc", [LOCN, 128], BF16, kind="Internal")
    out = nc.dram_tensor("out", [ngroups * GS * SN, HD], F32,
                         kind="ExternalOutput")

    with tile.TileContext(nc) as tc:
        with tc.tile_pool(name="const", bufs=1) as const, \
             tc.tile_pool(name="fp", bufs=3) as fpool, \
             tc.tile_pool(name="st1", bufs=4) as st1p, \
             tc.tile_pool(name="gg", bufs=3) as gp, \
             tc.tile_pool(name="ee", bufs=3) as ep, \
             tc.tile_pool(name="pc", bufs=2) as pcp, \
             tc.tile_pool(name="pp", bufs=2) as ppool, \
             tc.tile_pool(name="ux", bufs=3) as uxp, \
             tc.tile_pool(name="mm", bufs=2) as mxp, \
             tc.tile_pool(name="rr", bufs=8) as rp, \
             tc.tile_pool(name="so", bufs=4) as sop, \
             tc.tile_pool(name="ix", bufs=3) as ixp:
            # ---- weight prep: W_aug = [fc_w | W_l | W_r] (fp32 -> bf16) ----
            w_aug = const.tile([F, TCOLS], F32)
            nc.sync.dma_start(out=w_aug[:, 0:HD], in_=fcw[:, :])
            attn_sb = const.tile([1, 2 * HD], F32)
            nc.sync.dma_start(out=attn_sb[:], in_=attn[:, :])
            ab = const.tile([F, 2 * HD], F32)
            nc.gpsimd.partition_broadcast(ab[:], attn_sb[:])
            tmp = const.tile([F, 2 * HD], F32)
            nc.vector.tensor_tensor(
                out=tmp[:].rearrange("p (t w) -> p t w", t=2),
                in0=w_aug[:, None, 0:HD].broadcast_to([F, 2, HD]),
                in1=ab[:].rearrange("p (t w) -> p t w", t=2),
                op=AOT.mult,
            )
            nc.vector.tensor_reduce(
                w_aug[:, HD:HD + 2 * H].rearrange("p (t h) -> p t h", t=2),
                tmp[:].rearrange("p (t h d) -> p t h d", t=2, h=H),
                mybir.AxisListType.X,
                AOT.add,
            )
            w_bf = const.tile([F, TCOLS], BF16)
            nc.vector.tensor_copy(out=w_bf[:], in_=w_aug[:])

            # ---- phase 1b: er_loc for the local dst range ----
            with tc.tile_pool(name="p1b", bufs=1) as p1b, \
                 tc.tile_pool(name="p1bps", bufs=8, space="PSUM") as p1bps:
                fl = p1b.tile([F, LOCN], BF16)
                nc.sync.dma_start(out=fl[:], in_=featL[:, :])
                ntl = LOCN // 128
                erst = p1b.tile([128, ntl * H], BF16)
                for j in range(ntl):
                    ps = p1bps.tile([128, H], F32)
                    nc.tensor.matmul(
                        out=ps[:], lhsT=fl[:, j * 128:(j + 1) * 128],
                        rhs=w_bf[:, HD + H:HD + 2 * H],
                        start=True, stop=True)
                    nc.scalar.activation(
                        out=erst[:, j * H:(j + 1) * H], in_=ps[:],
                        func=ACT.Copy)
                nc.sync.dma_start(
                    out=erloc[:, 0:H].rearrange("(j p) c -> p j c", p=128),
                    in_=erst[:].rearrange("p (j c) -> p j c", c=H))

            # ---- phase 1: T2 = [feat @ W_aug] in bf16 256-elem rows ----
            with tc.tile_pool(name="p1ps", bufs=8, space="PSUM") as p1ps:
                tpw = wch // 128
                for w in range(npad // wch):
                    fsb = fpool.tile([F, wch], BF16)
                    nc.sync.dma_start(
                        out=fsb[:], in_=featT[:, w * wch:(w + 1) * wch])
                    for grp in range(tpw // g1):
                        stg = st1p.tile([F, g1 * TROW], BF16)
                        for j in range(g1):
                            ps = p1ps.tile([128, TCOLS], F32)
                            col0 = (grp * g1 + j) * 128
                            nc.tensor.matmul(
                                out=ps[:],
                                lhsT=fsb[:, col0:col0 + 128],
                                rhs=w_bf[:],
                                start=True, stop=True,
                            )
                            if j % 2 == 0:
                                nc.vector.tensor_copy(
                                    out=stg[:, j * TROW:j * TROW + TCOLS],
                                    in_=ps[:])
                            else:
                                nc.scalar.activation(
                                    out=stg[:, j * TROW:j * TROW + TCOLS],
                                    in_=ps[:], func=ACT.Copy)
                        t0 = w * tpw + grp * g1
                        r0, r1 = t0 * 128, (t0 + g1) * 128
                        j0 = 0
                        while r0 < r1:
                            b = r0 // BANK
                            seg = min(r1, (b + 1) * BANK) - r0
                            nj = seg // 128
                            nc.sync.dma_start(
                                out=T2b[b][r0 - b * BANK:
                                           r0 - b * BANK + seg, :].rearrange(
                                    "(j p) c -> p j c", j=nj),
                                in_=stg[:, j0 * TROW:(j0 + nj) * TROW]
                                .rearrange("p (j c) -> p j c", j=nj),
                            )
                            r0 += seg
                            j0 += nj

            # ---- phase 2: edge processing ----
            with tc.tile_pool(name="erps", bufs=2, space="PSUM") as erpsp, \
                 tc.tile_pool(name="p2ps", bufs=5, space="PSUM") as p2ps:
                for g in range(ngroups):
                    x0 = g * IXC
                    ix = ixp.tile([128, IXC], I16)
                    nc.sync.dma_start(
                        out=ix[:], in_=eidx[:, x0:x0 + IXC])
                    gt = gp.tile([CHE, GCH * TROW], BF16)
                    gv = gt[:].rearrange("p (c e) -> p c e", e=TROW)
                    for b in range(NB):
                        nc.gpsimd.dma_gather(
                            gv[:, b * GS * CB:(b + 1) * GS * CB, :],
                            T2b[b][:, :],
                            ix[:, b * 64:(b + 1) * 64],
                            GS * CB * CHE, GS * CB * CHE, TROW,
                            queue_num=(g + b) % 4)

                    ers = ep.tile([CHE, GS * 128], BF16)
                    nc.gpsimd.dma_gather(
                        ers[:].rearrange("p (c e) -> p c e", e=128),
                        erloc[:, :], ix[:, NB * 64:IXC],
                        GS * SN, GS * SN, 128, queue_num=g % 4)
                    erv = ers[:].rearrange("p (c e) -> p c e", e=128)

                    # er expansion: slot -> edge via host-built Pc one-hots
                    Pc = pcp.tile([SN, GED], BF16)
                    nc.sync.dma_start(
                        out=Pc[:], in_=pcd[:, g * GED:(g + 1) * GED])
                    erx_ps = erpsp.tile([CHE, GCH * H], F32)
                    for c in range(GCH):
                        s = (c % (GS * CB)) // CB
                        nc.tensor.matmul(
                            out=erx_ps[:, c * H:(c + 1) * H],
                            lhsT=Pc[:, c * CHE:(c + 1) * CHE],
                            rhs=erv[:, s, 0:H],
                            start=True, stop=True,
                        )
                    erx = ep.tile([CHE, GCH * H], BF16, tag="erx")
                    nc.scalar.activation(out=erx[:], in_=erx_ps[:],
                                         func=ACT.Copy)

                    u = uxp.tile([CHE, GCH * H], F32, tag="u")
                    nc.vector.tensor_tensor(
                        out=u[:].rearrange("p (c h) -> p c h", h=H),
                        in0=gv[:, :, HD:HD + H],
                        in1=erx[:].rearrange("p (c h) -> p c h", h=H),
                        op=AOT.add,
                    )
                    u2 = uxp.tile([CHE, GCH * H], F32, tag="u2")
                    nc.vector.scalar_tensor_tensor(
                        out=u2[:], in0=u[:], scalar=NEG, in1=u[:],
                        op0=AOT.mult, op1=AOT.max)
                    ex = uxp.tile([CHE, GCH * H], BF16, tag="ex")
                    nc.scalar.activation(out=ex[:], in_=u2[:], func=ACT.Exp)
                    exv = ex[:].rearrange("p (c h) -> p c h", h=H)

                    P_t = ppool.tile([CHE, GCH * SN], BF16)
                    nc.scalar.dma_start(
                        out=P_t[:],
                        in_=ptd[:, g * GCH * SN:(g + 1) * GCH * SN])

                    mx = mxp.tile([CHE, GCH * ML], BF16)
                    mv = mx[:].rearrange("p (c w) -> p c w", w=ML)
                    nc.scalar.activation(out=mv[:, :, HD:HD + H], in_=exv,
                                         func=ACT.Copy)
                    nc.vector.tensor_tensor(
                        out=mv[:, :, 0:HD].rearrange(
                            "p c (h d) -> p c h d", h=H),
                        in0=gv[:, :, 0:HD].rearrange(
                            "p c (h d) -> p c h d", h=H),
                        in1=exv[:, :, :, None].broadcast_to([CHE, GCH, H, D]),
                        op=AOT.mult,
                    )

                    for s in range(GS):
                        ps = p2ps.tile([SN, ML], F32)
                        for b in range(NB):
                            for i in range(CB):
                                c = b * GS * CB + s * CB + i
                                nc.tensor.matmul(
                                    out=ps[:],
                                    lhsT=P_t[:, c * SN:(c + 1) * SN],
                                    rhs=mx[:, c * ML:(c + 1) * ML],
                                    start=(b == 0 and i == 0),
                                    stop=(b == NB - 1 and i == CB - 1),
                                )
                        r0 = rp.tile([SN, H], F32, tag="r0")
                        nc.vector.tensor_scalar_max(r0[:], ps[:, HD:HD + H],
                                                    1e-30)
                        r1 = rp.tile([SN, H], F32, tag="r1")
                        nc.vector.reciprocal(r1[:], r0[:])
                        stg = sop.tile([SN, HD], F32)
                        nc.vector.tensor_tensor(
                            out=stg[:].rearrange("p (h d) -> p h d", h=H),
                            in0=ps[:, 0:HD].rearrange("p (h d) -> p h d", h=H),
                            in1=r1[:, :, None].broadcast_to([SN, H, D]),
                            op=AOT.mult,
                        )
                        nc.sync.dma_start(
                            out=out[(g * GS + s) * SN:(g * GS + s + 1) * SN, :],
                            in_=stg[:])

    nc.compile()
    return nc


_NC_CACHE = {}
LAST_RESULTS = None


def _get_program(ngroups):
    if ngroups not in _NC_CACHE:
        _NC_CACHE[ngroups] = _build(ngroups)
    return _NC_CACHE[ngroups]


def kernel(feat, fc_w, attn_l, attn_r, src, dst):
    import ml_dtypes
    from concourse.bass_utils import run_bass_kernel_spmd

    feat = np.asarray(feat, dtype=np.float32)
    fc_w = np.ascontiguousarray(np.asarray(fc_w, dtype=np.float32))
    attn_l = np.asarray(attn_l, dtype=np.float32)
    attn_r = np.asarray(attn_r, dtype=np.float32)
    src = np.asarray(src).astype(np.int64)
    dst = np.asarray(dst).astype(np.int64)

    edatas, node_maps, ngroups = _pack(src, dst)

    featT = np.zeros((F, NPAD), np.float32)
    featT[:, :N] = feat.T
    featT_bf = featT.astype(ml_dtypes.bfloat16)
    attn = np.concatenate(
        [attn_l.reshape(-1), attn_r.reshape(-1)]).reshape(1, 2 * HD)
    attn = np.ascontiguousarray(attn.astype(np.float32))

    nc = _get_program(ngroups)
    in_maps = [
        {"featT": featT_bf,
         "featL": np.ascontiguousarray(featT_bf[:, k * LOCN:(k + 1) * LOCN]),
         "fcw": fc_w, "attn": attn,
         "eidx": edatas[k]["eidx"], "ptd": edatas[k]["ptd"],
         "pcd": edatas[k]["pcd"]}
        for k in range(NCORES)
    ]
    res = run_bass_kernel_spmd(nc, in_maps, core_ids=list(range(NCORES)))
    global LAST_RESULTS
    LAST_RESULTS = res

    outf = np.zeros((N, HD), np.float32)
    for k in range(NCORES):
        o = np.asarray(res.results[k]["out"])
        nm = node_maps[k]
        m = nm >= 0
        outf[nm[m]] = o[m]
    return outf


# revision 24
# speedup vs baseline: 1.1927x; 1.1927x over previous
"""GAT message-passing kernel for Trainium2 (8 NeuronCores, Bass/Tile).

Strategy (edge/graph parallelism, per the sharding hint):
  - Host: shard dst nodes into 8 fixed ranges of LOCN=12544. Per core, pack
    dst nodes into supertiles of <=128 node slots; each supertile holds 16
    edge chunks of 128 slots: 4 chunks per src bank (src space split into 4
    banks of 25088 rows so gather indices fit int16 for the SWDGE dma_gather
    instruction). Gather groups of GS=2 supertiles make each per-bank gather
    exactly 1024 indices (the SWDGE descriptor-ring capacity).
  - Device phase 1 (replicated): T2[n] = [feat@fc_w | el | er] as bf16 rows
    of 256 elems (512B stride). Phase 1b computes a per-core er table
    (er_loc) for the LOCAL dst range from a host-sliced feat input, keeping
    the SPMD program uniform while er-gather indices stay dst-local.
  - Device phase 2, per group (32 chunks, 4096 edge slots): 4 batched
    dma_gathers (one per src bank, 1024 rows each, round-robin over 4 SWDGE
    queues) fetch [feat_src|el]; 1 small dma_gather fetches er per NODE SLOT
    (256 rows); er is expanded slot->edge with one-hot matmuls (Pc) into a
    single PSUM bank. Compute ex = exp(leaky_relu(el+er)); build one-hot
    edge->slot P_t; 16 accumulating bf16 matmuls per supertile
    P^T @ [feat*ex | ex] into a [128 slots, 132] PSUM tile; divide by the
    summed ex (segment softmax denominator) and stream out rows.
"""

import numpy as np

# ---------------- problem constants (hardcoded; kernel.py is self-contained) ---
N = 100000
F = 128           # input feature dim (= contraction dim)
H = 4             # heads
D = 32            # dim per head
HD = H * D        # 128
TCOLS = F + 2 * H  # 136 = feat_src(128) + el(4) + er(4)
TROW = 256        # bf16 elems per T2 row (512B)
ML = HD + H       # 132 = msg cols + ex cols
NEG = 0.2
NCORES = 8

# ---------------- device tiling parameters ------------------------------------
NPAD = 100352     # node rows padded (= 8*LOCN = 4*BANK)
LOCN = NPAD // 8  # dst nodes per core = 12544
BANK = NPAD // 4  # src rows per bank = 25088
NB = 4            # src banks
SN = 128          # node slots per supertile
CB = 4            # chunks per bank per supertile
CHE = 128         # edges per chunk
GS = 2            # supertiles per gather group
GCH = NB * GS * CB  # chunks per group = 32
GED = GCH * CHE   # edge slots per group = 4096
SEG_PAD = SN      # seg value for padding edge slots (no one-hot match)
IXC = NB * 64 + 16  # eidx cols per group: 4 banks x 64 + 16 er-slot cols

# phase-1 layout
WCH = 2048        # featT columns per DMA load (16 tiles)
G1 = 8            # node tiles per T2 write


def _wrap16(idx):
    """[n] int -> [128, n//16] int16 in the 16-wrapped, core-replicated layout."""
    n = idx.shape[0]
    w = idx.astype(np.int16).reshape(n // 16, 16).T
    return np.tile(w, (8, 1))


def _pack(src, dst):
    """Host-side index preprocessing.

    Returns (per-core dicts of eidx/segd/segT arrays, node_maps, ngroups).
    """
    import ml_dtypes

    src = np.asarray(src, np.int64)
    dst = np.asarray(dst, np.int64)
    order = np.argsort(dst, kind="stable")
    s_src = src[order]
    s_dst = dst[order]
    core_of = s_dst // LOCN
    cuts = np.searchsorted(core_of, np.arange(NCORES + 1))

    packs = []
    for k in range(NCORES):
        lo = k * LOCN
        e0, e1 = cuts[k], cuts[k + 1]
        ksrc = s_src[e0:e1]
        nloc = s_dst[e0:e1] - lo
        kbank = ksrc // BANK

        degb = np.zeros((LOCN, NB), np.int64)
        np.add.at(degb, (nloc, kbank), 1)

        # greedy supertile packing in dst order
        sup_id = np.zeros(LOCN, np.int64)
        node_slot = np.zeros(LOCN, np.int64)
        cur, cnt_n = 0, 0
        cnt_b = np.zeros(NB, np.int64)
        cap = CB * CHE
        for n in range(LOCN):
            dnb = degb[n]
            if cnt_n >= SN or np.any(cnt_b + dnb > cap):
                cur += 1
                cnt_n = 0
                cnt_b[:] = 0
            sup_id[n] = cur
            node_slot[n] = cnt_n
            cnt_n += 1
            cnt_b += dnb
        packs.append((ksrc, nloc, kbank, sup_id, node_slot, cur + 1))

    nsup_max = max(p[5] for p in packs)
    ngroups = (nsup_max + GS - 1) // GS
    nsup_pad = ngroups * GS
    nchunk = ngroups * GCH

    edatas = []
    node_maps = []
    for k in range(NCORES):
        ksrc, nloc, kbank, sup_id, node_slot, nsup = packs[k]
        esup = sup_id[nloc]
        eslot = node_slot[nloc]
        # position within (sup, bank) group, preserving dst order
        key = esup * NB + kbank
        o = np.argsort(key, kind="stable")
        sk = key[o]
        starts = np.searchsorted(sk, np.arange(nsup * NB))
        cnts = np.diff(np.append(starts, len(sk)))
        pos = np.empty(len(sk), np.int64)
        pos[o] = np.arange(len(sk)) - np.repeat(starts, cnts)
        cb = pos // CHE
        assert cb.max(initial=0) < CB
        p = pos % CHE
        gc = ((esup // GS) * GCH + kbank * (GS * CB)
              + (esup % GS) * CB + cb)

        srcloc = np.zeros((CHE, nchunk), np.int16)
        srcloc[p, gc] = (ksrc - kbank * BANK).astype(np.int16)
        # host-built one-hot matrices (edge->slot and slot->edge), bf16
        pth = np.zeros((CHE, nchunk * SN), ml_dtypes.float8_e4m3)
        pth[p, gc * SN + eslot] = 1
        pch = np.zeros((SN, nchunk * CHE), ml_dtypes.float8_e4m3)
        pch[eslot, gc * CHE + p] = 1

        # er-slot node ids (dst-local), one per supertile slot
        nid = np.zeros(nsup_pad * SN, np.int16)
        nid[sup_id * SN + node_slot] = np.arange(LOCN).astype(np.int16)

        # eidx per group: [bank0 | bank1 | bank2 | bank3 | er-slots]
        eidx = np.zeros((128, ngroups * IXC), np.int16)
        for g in range(ngroups):
            c0 = g * GCH
            x0 = g * IXC
            for b in range(NB):
                run = srcloc[:, c0 + b * GS * CB:c0 + (b + 1) * GS * CB]
                eidx[:, x0 + b * 64:x0 + (b + 1) * 64] = \
                    _wrap16(run.T.reshape(-1))
            eidx[:, x0 + NB * 64:x0 + IXC] = \
                _wrap16(nid[g * GS * SN:(g + 1) * GS * SN])

        nm = np.full(nsup_pad * SN, -1, np.int64)
        nm[sup_id * SN + node_slot] = np.arange(LOCN) + k * LOCN
        nm[nm >= N] = -1
        edatas.append({"eidx": eidx, "ptd": pth, "pcd": pch})
        node_maps.append(nm)
    return edatas, node_maps, ngroups


def _build(ngroups, npad=NPAD, wch=WCH, g1=G1):
    """Build the per-core Bass program (identical across cores)."""
    import concourse.bacc as bacc
    import concourse.tile as tile
    import concourse.mybir as mybir

    F32 = mybir.dt.float32
    BF16 = mybir.dt.bfloat16
    I32 = mybir.dt.int32
    I16 = mybir.dt.int16
    FP8 = mybir.dt.float8e4
    AOT = mybir.AluOpType
    ACT = mybir.ActivationFunctionType

    nchunk = ngroups * GCH

    nc = bacc.Bacc("TRN2", target_bir_lowering=False, debug=False,
                   num_swdge_queues=4)
    featT = nc.dram_tensor("featT", [F, npad], BF16, kind="ExternalInput")
    featL = nc.dram_tensor("featL", [F, LOCN], BF16, kind="ExternalInput")
    fcw = nc.dram_tensor("fcw", [F, HD], F32, kind="ExternalInput")
    attn = nc.dram_tensor("attn", [1, 2 * HD], F32, kind="ExternalInput")
    eidx = nc.dram_tensor("eidx", [128, ngroups * IXC], I16,
                          kind="ExternalInput")
    ptd = nc.dram_tensor("ptd", [CHE, nchunk * SN], FP8,
                         kind="ExternalInput")
    pcd = nc.dram_tensor("pcd", [SN, nchunk * CHE], FP8,
                         kind="ExternalInput")
    T2b = [nc.dram_tensor(f"T2b{b}", [BANK, TROW], BF16, kind="Internal")
           for b in range(NB)]
    erloc = nc.dram_tensor("erloc", [LOCN, 128], BF16, kind="Internal")
    out = nc.dram_tensor("out", [ngroups * GS * SN, HD], F32,
                         kind="ExternalOutput")

    with tile.TileContext(nc) as tc:
        with tc.tile_pool(name="const", bufs=1) as const, \
             tc.tile_pool(name="fp", bufs=3) as fpool, \
             tc.tile_pool(name="st1", bufs=6) as st1p, \
             tc.tile_pool(name="gg", bufs=3) as gp, \
             tc.tile_pool(name="ee", bufs=3) as ep, \
             tc.tile_pool(name="pc", bufs=2) as pcp, \
             tc.tile_pool(name="pp", bufs=2) as ppool, \
             tc.tile_pool(name="ux", bufs=3) as uxp, \
             tc.tile_pool(name="mm", bufs=2) as mxp, \
             tc.tile_pool(name="rr", bufs=8) as rp, \
             tc.tile_pool(name="so", bufs=4) as sop, \
             tc.tile_pool(name="ix", bufs=3) as ixp:
            # ---- weight prep: W_aug = [fc_w | W_l | W_r] (fp32 -> bf16) ----
            w_aug = const.tile([F, TCOLS], F32)
            nc.sync.dma_start(out=w_aug[:, 0:HD], in_=fcw[:, :])
            attn_sb = const.tile([1, 2 * HD], F32)
            nc.sync.dma_start(out=attn_sb[:], in_=attn[:, :])
            ab = const.tile([F, 2 * HD], F32)
            nc.gpsimd.partition_broadcast(ab[:], attn_sb[:])
            tmp = const.tile([F, 2 * HD], F32)
            nc.vector.tensor_tensor(
                out=tmp[:].rearrange("p (t w) -> p t w", t=2),
                in0=w_aug[:, None, 0:HD].broadcast_to([F, 2, HD]),
                in1=ab[:].rearrange("p (t w) -> p t w", t=2),
                op=AOT.mult,
            )
            nc.vector.tensor_reduce(
                w_aug[:, HD:HD + 2 * H].rearrange("p (t h) -> p t h", t=2),
                tmp[:].rearrange("p (t h d) -> p t h d", t=2, h=H),
                mybir.AxisListType.X,
                AOT.add,
            )
            w_bf = const.tile([F, TCOLS], BF16)
            nc.vector.tensor_copy(out=w_bf[:], in_=w_aug[:])

            # ---- phase 1b: er_loc for the local dst range ----
            with tc.tile_pool(name="p1b", bufs=1) as p1b, \
                 tc.tile_pool(name="p1bps", bufs=8, space="PSUM") as p1bps:
                fl = p1b.tile([F, LOCN], BF16)
                nc.sync.dma_start(out=fl[:], in_=featL[:, :])
                ntl = LOCN // 128
                erst = p1b.tile([128, ntl * H], BF16)
                for j in range(ntl):
                    ps = p1bps.tile([128, H], F32)
                    nc.tensor.matmul(
                        out=ps[:], lhsT=fl[:, j * 128:(j + 1) * 128],
                        rhs=w_bf[:, HD + H:HD + 2 * H],
                        start=True, stop=True)
                    nc.scalar.activation(
                        out=erst[:, j * H:(j + 1) * H], in_=ps[:],
                        func=ACT.Copy)
                nc.sync.dma_start(
                    out=erloc[:, 0:H].rearrange("(j p) c -> p j c", p=128),
                    in_=erst[:].rearrange("p (j c) -> p j c", c=H))

            # ---- phase 1: T2 = [feat @ W_aug] in bf16 256-elem rows ----
            with tc.tile_pool(name="p1ps", bufs=8, space="PSUM") as p1ps:
                tpw = wch // 128
                for w in range(npad // wch):
                    fsb = fpool.tile([F, wch], BF16)
                    nc.sync.dma_start(
                        out=fsb[:], in_=featT[:, w * wch:(w + 1) * wch])
                    for grp in range(tpw // g1):
                        stg = st1p.tile([F, g1 * TROW], BF16)
                        for j in range(g1):
                            ps = p1ps.tile([128, TCOLS], F32)
                            col0 = (grp * g1 + j) * 128
                            nc.tensor.matmul(
                                out=ps[:],
                                lhsT=fsb[:, col0:col0 + 128],
                                rhs=w_bf[:],
                                start=True, stop=True,
                            )
                            if j % 2 == 0:
                                nc.vector.tensor_copy(
                                    out=stg[:, j * TROW:j * TROW + TCOLS],
                                    in_=ps[:])
                            else:
                                nc.scalar.activation(
                                    out=stg[:, j * TROW:j * TROW + TCOLS],
                                    in_=ps[:], func=ACT.Copy)
                        t0 = w * tpw + grp * g1
                        r0, r1 = t0 * 128, (t0 + g1) * 128
                        j0 = 0
                        while r0 < r1:
                            b = r0 // BANK
                            seg = min(r1, (b + 1) * BANK) - r0
                            nj = seg // 128
                            nc.sync.dma_start(
                                out=T2b[b][r0 - b * BANK:
                                           r0 - b * BANK + seg, :].rearrange(
                                    "(j p) c -> p j c", j=nj),
                                in_=stg[:, j0 * TROW:(j0 + nj) * TROW]
                                .rearrange("p (j c) -> p j c", j=nj),
                            )
                            r0 += seg
                            j0 += nj

            # ---- phase 2: edge processing ----
            with tc.tile_pool(name="erps", bufs=2, space="PSUM") as erpsp, \
                 tc.tile_pool(name="p2ps", bufs=5, space="PSUM") as p2ps:
                for g in range(ngroups):
                    x0 = g * IXC
                    ix = ixp.tile([128, IXC], I16)
                    nc.sync.dma_start(
                        out=ix[:], in_=eidx[:, x0:x0 + IXC])
                    gt = gp.tile([CHE, GCH * TROW], BF16)
                    gv = gt[:].rearrange("p (c e) -> p c e", e=TROW)
                    for b in range(NB):
                        nc.gpsimd.dma_gather(
                            gv[:, b * GS * CB:(b + 1) * GS * CB, :],
                            T2b[b][:, :],
                            ix[:, b * 64:(b + 1) * 64],
                            GS * CB * CHE, GS * CB * CHE, TROW,
                            queue_num=(5 * g + b) % 4)

                    ers = ep.tile([CHE, GS * 128], BF16)
                    nc.gpsimd.dma_gather(
                        ers[:].rearrange("p (c e) -> p c e", e=128),
                        erloc[:, :], ix[:, NB * 64:IXC],
                        GS * SN, GS * SN, 128, queue_num=(5 * g + 4) % 4)
                    erv = ers[:].rearrange("p (c e) -> p c e", e=128)

                    # er expansion: slot -> edge via host-built Pc one-hots
                    Pc = pcp.tile([SN, GED], FP8)
                    nc.scalar.dma_start(
                        out=Pc[:], in_=pcd[:, g * GED:(g + 1) * GED])
                    erx_ps = erpsp.tile([CHE, GCH * H], F32)
                    for c in range(GCH):
                        s = (c % (GS * CB)) // CB
                        nc.tensor.matmul(
                            out=erx_ps[:, c * H:(c + 1) * H],
                            lhsT=Pc[:, c * CHE:(c + 1) * CHE],
                            rhs=erv[:, s, 0:H],
                            start=True, stop=True,
                        )
                    erx = ep.tile([CHE, GCH * H], BF16, tag="erx")
                    nc.scalar.activation(out=erx[:], in_=erx_ps[:],
                                         func=ACT.Copy)

                    u = uxp.tile([CHE, GCH * H], F32, tag="u")
                    nc.vector.tensor_tensor(
                        out=u[:].rearrange("p (c h) -> p c h", h=H),
                        in0=gv[:, :, HD:HD + H],
                        in1=erx[:].rearrange("p (c h) -> p c h", h=H),
                        op=AOT.add,
                    )
                    u2 = uxp.tile([CHE, GCH * H], F32, tag="u2")
                    nc.vector.scalar_tensor_tensor(
                        out=u2[:], in0=u[:], scalar=NEG, in1=u[:],
                        op0=AOT.mult, op1=AOT.max)
                    ex = uxp.tile([CHE, GCH * H], BF16, tag="ex")
                    nc.scalar.activation(out=ex[:], in_=u2[:], func=ACT.Exp)
                    exv = ex[:].rearrange("p (c h) -> p c h", h=H)

                    P_t = ppool.tile([CHE, GCH * SN], FP8)
                    nc.scalar.dma_start(
                        out=P_t[:],
                        in_=ptd[:, g * GCH * SN:(g + 1) * GCH * SN])

                    mx = mxp.tile([CHE, GCH * ML], BF16)
                    mv = mx[:].rearrange("p (c w) -> p c w", w=ML)
                    nc.scalar.activation(out=mv[:, :, HD:HD + H], in_=exv,
                                         func=ACT.Copy)
                    nc.vector.tensor_tensor(
                        out=mv[:, :, 0:HD].rearrange(
                            "p c (h d) -> p c h d", h=H),
                        in0=gv[:, :, 0:HD].rearrange(
                            "p c (h d) -> p c h d", h=H),
                        in1=exv[:, :, :, None].broadcast_to([CHE, GCH, H, D]),
                        op=AOT.mult,
                    )

                    for s in range(GS):
                        ps = p2ps.tile([SN, ML], F32)
                        for b in range(NB):
                            for i in range(CB):
                                c = b * GS * CB + s * CB + i
                                nc.tensor.matmul(
                                    out=ps[:],
                                    lhsT=P_t[:, c * SN:(c + 1) * SN],
                                    rhs=mx[:, c * ML:(c + 1) * ML],
                                    start=(b == 0 and i == 0),
                                    stop=(b == NB - 1 and i == CB - 1),
                                )
                        r0 = rp.tile([SN, H], F32, tag="r0")
                        nc.vector.tensor_scalar_max(r0[:], ps[:, HD:HD + H],
                                                    1e-30)
                        r1 = rp.tile([SN, H], F32, tag="r1")
                        nc.vector.reciprocal(r1[:], r0[:])
                        stg = sop.tile([SN, HD], F32)
                        nc.vector.tensor_tensor(
                            out=stg[:].rearrange("p (h d) -> p h d", h=H),
                            in0=ps[:, 0:HD].rearrange("p (h d) -> p h d", h=H),
                            in1=r1[:, :, None].broadcast_to([SN, H, D]),
                            op=AOT.mult,
                        )
                        nc.sync.dma_start(
                            out=out[(g * GS + s) * SN:(g * GS + s + 1) * SN, :],
                            in_=stg[:])

    nc.compile()
    return nc


_NC_CACHE = {}
LAST_RESULTS = None


def _get_program(ngroups):
    if ngroups not in _NC_CACHE:
        _NC_CACHE[ngroups] = _build(ngroups)
    return _NC_CACHE[ngroups]


def kernel(feat, fc_w, attn_l, attn_r, src, dst):
    import ml_dtypes
    from concourse.bass_utils import run_bass_kernel_spmd

    feat = np.asarray(feat, dtype=np.float32)
    fc_w = np.ascontiguousarray(np.asarray(fc_w, dtype=np.float32))
    attn_l = np.asarray(attn_l, dtype=np.float32)
    attn_r = np.asarray(attn_r, dtype=np.float32)
    src = np.asarray(src).astype(np.int64)
    dst = np.asarray(dst).astype(np.int64)

    edatas, node_maps, ngroups = _pack(src, dst)

    featT = np.zeros((F, NPAD), np.float32)
    featT[:, :N] = feat.T
    featT_bf = featT.astype(ml_dtypes.bfloat16)
    attn = np.concatenate(
        [attn_l.reshape(-1), attn_r.reshape(-1)]).reshape(1, 2 * HD)
    attn = np.ascontiguousarray(attn.astype(np.float32))

    nc = _get_program(ngroups)
    in_maps = [
        {"featT": featT_bf,
         "featL": np.ascontiguousarray(featT_bf[:, k * LOCN:(k + 1) * LOCN]),
         "fcw": fc_w, "attn": attn,
         "eidx": edatas[k]["eidx"], "ptd": edatas[k]["ptd"],
         "pcd": edatas[k]["pcd"]}
        for k in range(NCORES)
    ]
    res = run_bass_kernel_spmd(nc, in_maps, core_ids=list(range(NCORES)))
    global LAST_RESULTS
    LAST_RESULTS = res

    outf = np.zeros((N, HD), np.float32)
    for k in range(NCORES):
        o = np.asarray(res.results[k]["out"])
        nm = node_maps[k]
        m = nm >= 0
        outf[nm[m]] = o[m]
    return outf


# revision 32
# speedup vs baseline: 1.2859x; 1.0782x over previous
"""GAT message-passing kernel for Trainium2 (8 NeuronCores, Bass/Tile).

Strategy (edge/graph parallelism, per the sharding hint):
  - Host: shard dst nodes into 8 fixed ranges of LOCN=12544. Per core, pack
    dst nodes into supertiles of <=128 node slots; each supertile holds 16
    edge chunks of 128 slots: 4 chunks per src bank (src space split into 4
    banks of 25088 rows so gather indices fit int16 for the SWDGE dma_gather
    instruction). Gather groups of GS=2 supertiles make each per-bank gather
    exactly 1024 indices (the SWDGE descriptor-ring capacity).
  - Device phase 1 (replicated): T2[n] = [feat@fc_w | el | er] as bf16 rows
    of 256 elems (512B stride). Phase 1b computes a per-core er table
    (er_loc) for the LOCAL dst range from a host-sliced feat input, keeping
    the SPMD program uniform while er-gather indices stay dst-local.
  - Device phase 2, per group (32 chunks, 4096 edge slots): 4 batched
    dma_gathers (one per src bank, 1024 rows each, round-robin over 4 SWDGE
    queues) fetch [feat_src|el]; 1 small dma_gather fetches er per NODE SLOT
    (256 rows); er is expanded slot->edge with one-hot matmuls (Pc) into a
    single PSUM bank. Compute ex = exp(leaky_relu(el+er)); build one-hot
    edge->slot P_t; 16 accumulating bf16 matmuls per supertile
    P^T @ [feat*ex | ex] into a [128 slots, 132] PSUM tile; divide by the
    summed ex (segment softmax denominator) and stream out rows.
"""

import numpy as np

# ---------------- problem constants (hardcoded; kernel.py is self-contained) ---
N = 100000
F = 128           # input feature dim (= contraction dim)
H = 4             # heads
D = 32            # dim per head
HD = H * D        # 128
TCOLS = F + 2 * H  # 136 = feat_src(128) + el(4) + er(4)
TROW = 256        # bf16 elems per T2 row (512B)
ML = HD + H       # 132 = msg cols + ex cols
NEG = 0.2
NCORES = 8

# ---------------- device tiling parameters ------------------------------------
NPAD = 100352     # node rows padded (= 8*LOCN = 4*BANK)
LOCN = NPAD // 8  # dst nodes per core = 12544
BANK = NPAD // 4  # src rows per bank = 25088
NB = 4            # src banks
SN = 128          # node slots per supertile
CB = 4            # chunks per bank per supertile
CHE = 128         # edges per chunk
GS = 2            # supertiles per gather group
GCH = NB * GS * CB  # chunks per group = 32
GED = GCH * CHE   # edge slots per group = 4096
SEG_PAD = SN      # seg value for padding edge slots (no one-hot match)
IXC = NB * 64 + 16  # eidx cols per group: 4 banks x 64 + 16 er-slot cols

# phase-1 layout
WCH = 2048        # featT columns per DMA load (16 tiles)
G1 = 8            # node tiles per T2 write


def _wrap16(idx):
    """[n] int -> [128, n//16] int16 in the 16-wrapped, core-replicated layout."""
    n = idx.shape[0]
    w = idx.astype(np.int16).reshape(n // 16, 16).T
    return np.tile(w, (8, 1))


# phase-1 writes T2 in blocks of NT tiles; within a block, partition p of
# matmul-tile j lands at row p*NT+j so the DRAM write is fully contiguous.
_P1_BLOCKS = [16] * 12 + [4]  # 12*16 + 4 = 196 tiles = one bank


def _row_of():
    """node id -> T2 row under the per-block (j,p)->(p*nt+j) permutation."""
    row = np.empty(NPAD, np.int64)
    for b in range(NB):
        off = 0
        for nt in _P1_BLOCKS:
            cnt = nt * 128
            idx = np.arange(cnt)
            j, p = idx // 128, idx % 128
            row[b * BANK + off + idx] = b * BANK + off + p * nt + j
            off += cnt
        assert off == BANK
    return row


def _pack(src, dst):
    """Host-side index preprocessing.

    Returns (per-core dicts of eidx/segd/segT arrays, node_maps, ngroups).
    """
    import ml_dtypes

    src = np.asarray(src, np.int64)
    dst = np.asarray(dst, np.int64)
    order = np.argsort(dst, kind="stable")
    s_src = src[order]
    s_dst = dst[order]
    core_of = s_dst // LOCN
    cuts = np.searchsorted(core_of, np.arange(NCORES + 1))

    packs = []
    for k in range(NCORES):
        lo = k * LOCN
        e0, e1 = cuts[k], cuts[k + 1]
        ksrc = s_src[e0:e1]
        nloc = s_dst[e0:e1] - lo
        kbank = ksrc // BANK

        degb = np.zeros((LOCN, NB), np.int64)
        np.add.at(degb, (nloc, kbank), 1)

        # greedy supertile packing in dst order
        sup_id = np.zeros(LOCN, np.int64)
        node_slot = np.zeros(LOCN, np.int64)
        cur, cnt_n = 0, 0
        cnt_b = np.zeros(NB, np.int64)
        cap = CB * CHE
        for n in range(LOCN):
            dnb = degb[n]
            if cnt_n >= SN or np.any(cnt_b + dnb > cap):
                cur += 1
                cnt_n = 0
                cnt_b[:] = 0
            sup_id[n] = cur
            node_slot[n] = cnt_n
            cnt_n += 1
            cnt_b += dnb
        packs.append((ksrc, nloc, kbank, sup_id, node_slot, cur + 1))

    nsup_max = max(p[5] for p in packs)
    ngroups = (nsup_max + GS - 1) // GS
    nsup_pad = ngroups * GS
    nchunk = ngroups * GCH

    edatas = []
    node_maps = []
    for k in range(NCORES):
        ksrc, nloc, kbank, sup_id, node_slot, nsup = packs[k]
        esup = sup_id[nloc]
        eslot = node_slot[nloc]
        # position within (sup, bank) group, preserving dst order
        key = esup * NB + kbank
        o = np.argsort(key, kind="stable")
        sk = key[o]
        starts = np.searchsorted(sk, np.arange(nsup * NB))
        cnts = np.diff(np.append(starts, len(sk)))
        pos = np.empty(len(sk), np.int64)
        pos[o] = np.arange(len(sk)) - np.repeat(starts, cnts)
        cb = pos // CHE
        assert cb.max(initial=0) < CB
        p = pos % CHE
        gc = ((esup // GS) * GCH + kbank * (GS * CB)
              + (esup % GS) * CB + cb)

        row_of = _row_of()
        srcloc = np.zeros((CHE, nchunk), np.int16)
        srcloc[p, gc] = (row_of[ksrc] - kbank * BANK).astype(np.int16)
        # host-built one-hot matrices (edge->slot and slot->edge), bf16
        pth = np.zeros((CHE, nchunk * SN), ml_dtypes.float8_e4m3)
        pth[p, gc * SN + eslot] = 1
        pch = np.zeros((SN, nchunk * CHE), ml_dtypes.float8_e4m3)
        pch[eslot, gc * CHE + p] = 1

        # er-slot node ids (dst-local), one per supertile slot
        nid = np.zeros(nsup_pad * SN, np.int16)
        nid[sup_id * SN + node_slot] = np.arange(LOCN).astype(np.int16)

        # eidx per group: [bank0 | bank1 | bank2 | bank3 | er-slots]
        eidx = np.zeros((128, ngroups * IXC), np.int16)
        for g in range(ngroups):
            c0 = g * GCH
            x0 = g * IXC
            for b in range(NB):
                run = srcloc[:, c0 + b * GS * CB:c0 + (b + 1) * GS * CB]
                eidx[:, x0 + b * 64:x0 + (b + 1) * 64] = \
                    _wrap16(run.T.reshape(-1))
            eidx[:, x0 + NB * 64:x0 + IXC] = \
                _wrap16(nid[g * GS * SN:(g + 1) * GS * SN])

        nm = np.full(nsup_pad * SN, -1, np.int64)
        nm[sup_id * SN + node_slot] = np.arange(LOCN) + k * LOCN
        nm[nm >= N] = -1
        edatas.append({"eidx": eidx, "ptd": pth, "pcd": pch})
        node_maps.append(nm)
    return edatas, node_maps, ngroups


def _build(ngroups, npad=NPAD, wch=WCH, g1=G1):
    """Build the per-core Bass program (identical across cores)."""
    import concourse.bacc as bacc
    import concourse.tile as tile
    import concourse.mybir as mybir

    F32 = mybir.dt.float32
    BF16 = mybir.dt.bfloat16
    I32 = mybir.dt.int32
    I16 = mybir.dt.int16
    FP8 = mybir.dt.float8e4
    AOT = mybir.AluOpType
    ACT = mybir.ActivationFunctionType

    nchunk = ngroups * GCH

    nc = bacc.Bacc("TRN2", target_bir_lowering=False, debug=False,
                   num_swdge_queues=4)
    featT = nc.dram_tensor("featT", [F, npad], BF16, kind="ExternalInput")
    featL = nc.dram_tensor("featL", [F, LOCN], BF16, kind="ExternalInput")
    fcw = nc.dram_tensor("fcw", [F, HD], F32, kind="ExternalInput")
    attn = nc.dram_tensor("attn", [1, 2 * HD], F32, kind="ExternalInput")
    eidx = nc.dram_tensor("eidx", [128, ngroups * IXC], I16,
                          kind="ExternalInput")
    ptd = nc.dram_tensor("ptd", [CHE, nchunk * SN], FP8,
                         kind="ExternalInput")
    pcd = nc.dram_tensor("pcd", [SN, nchunk * CHE], FP8,
                         kind="ExternalInput")
    T2b = [nc.dram_tensor(f"T2b{b}", [BANK, TROW], BF16, kind="Internal")
           for b in range(NB)]
    erloc = nc.dram_tensor("erloc", [LOCN, 128], BF16, kind="Internal")
    out = nc.dram_tensor("out", [ngroups * GS * SN, HD], F32,
                         kind="ExternalOutput")

    with tile.TileContext(nc) as tc:
        with tc.tile_pool(name="const", bufs=1) as const, \
             tc.tile_pool(name="fp", bufs=3) as fpool, \
             tc.tile_pool(name="st1", bufs=4) as st1p, \
             tc.tile_pool(name="gg", bufs=3) as gp, \
             tc.tile_pool(name="ee", bufs=3) as ep, \
             tc.tile_pool(name="pc", bufs=2) as pcp, \
             tc.tile_pool(name="pp", bufs=2) as ppool, \
             tc.tile_pool(name="ux", bufs=3) as uxp, \
             tc.tile_pool(name="mm", bufs=2) as mxp, \
             tc.tile_pool(name="rr", bufs=8) as rp, \
             tc.tile_pool(name="so", bufs=4) as sop, \
             tc.tile_pool(name="ix", bufs=3) as ixp:
            # ---- weight prep: W_aug = [fc_w | W_l | W_r] (fp32 -> bf16) ----
            w_aug = const.tile([F, TCOLS], F32)
            nc.sync.dma_start(out=w_aug[:, 0:HD], in_=fcw[:, :])
            attn_sb = const.tile([1, 2 * HD], F32)
            nc.sync.dma_start(out=attn_sb[:], in_=attn[:, :])
            ab = const.tile([F, 2 * HD], F32)
            nc.gpsimd.partition_broadcast(ab[:], attn_sb[:])
            tmp = const.tile([F, 2 * HD], F32)
            nc.vector.tensor_tensor(
                out=tmp[:].rearrange("p (t w) -> p t w", t=2),
                in0=w_aug[:, None, 0:HD].broadcast_to([F, 2, HD]),
                in1=ab[:].rearrange("p (t w) -> p t w", t=2),
                op=AOT.mult,
            )
            nc.vector.tensor_reduce(
                w_aug[:, HD:HD + 2 * H].rearrange("p (t h) -> p t h", t=2),
                tmp[:].rearrange("p (t h d) -> p t h d", t=2, h=H),
                mybir.AxisListType.X,
                AOT.add,
            )
            w_bf = const.tile([F, TCOLS], BF16)
            nc.vector.tensor_copy(out=w_bf[:], in_=w_aug[:])

            # ---- phase 1b: er_loc for the local dst range ----
            with tc.tile_pool(name="p1b", bufs=1) as p1b, \
                 tc.tile_pool(name="p1bps", bufs=8, space="PSUM") as p1bps:
                fl = p1b.tile([F, LOCN], BF16)
                nc.sync.dma_start(out=fl[:], in_=featL[:, :])
                ntl = LOCN // 128
                erst = p1b.tile([128, ntl * H], BF16)
                for j in range(ntl):
                    ps = p1bps.tile([128, H], F32)
                    nc.tensor.matmul(
                        out=ps[:], lhsT=fl[:, j * 128:(j + 1) * 128],
                        rhs=w_bf[:, HD + H:HD + 2 * H],
                        start=True, stop=True)
                    nc.scalar.activation(
                        out=erst[:, j * H:(j + 1) * H], in_=ps[:],
                        func=ACT.Copy)
                nc.sync.dma_start(
                    out=erloc[:, 0:H].rearrange("(j p) c -> p j c", p=128),
                    in_=erst[:].rearrange("p (j c) -> p j c", c=H))

            # ---- phase 1: T2 = [feat @ W_aug] in bf16 256-elem rows ----
            # per-block permuted layout: row p*nt+j <- (tile j, partition p),
            # so each block's T2 write is one fully contiguous DMA.
            with tc.tile_pool(name="p1ps", bufs=8, space="PSUM") as p1ps:
                for b in range(NB):
                    off = 0
                    for nt in _P1_BLOCKS:
                        fsb = fpool.tile([F, nt * 128], BF16, tag=f"f{nt}")
                        nc.sync.dma_start(
                            out=fsb[:],
                            in_=featT[:, b * BANK + off:
                                      b * BANK + off + nt * 128])
                        stg = st1p.tile([F, nt * TROW], BF16, tag=f"s{nt}")
                        for j in range(nt):
                            ps = p1ps.tile([128, TCOLS], F32)
                            nc.tensor.matmul(
                                out=ps[:],
                                lhsT=fsb[:, j * 128:(j + 1) * 128],
                                rhs=w_bf[:],
                                start=True, stop=True,
                            )
                            if j % 2 == 0:
                                nc.vector.tensor_copy(
                                    out=stg[:, j * TROW:j * TROW + TCOLS],
                                    in_=ps[:])
                            else:
                                nc.scalar.activation(
                                    out=stg[:, j * TROW:j * TROW + TCOLS],
                                    in_=ps[:], func=ACT.Copy)
                        nc.sync.dma_start(
                            out=T2b[b][off:off + nt * 128, :].rearrange(
                                "(p j) c -> p j c", p=128),
                            in_=stg[:].rearrange("p (j c) -> p j c", j=nt),
                        )
                        off += nt * 128

            # ---- phase 2: edge processing ----
            with tc.tile_pool(name="erps", bufs=2, space="PSUM") as erpsp, \
                 tc.tile_pool(name="p2ps", bufs=5, space="PSUM") as p2ps:
                for g in range(ngroups):
                    x0 = g * IXC
                    ix = ixp.tile([128, IXC], I16)
                    nc.sync.dma_start(
                        out=ix[:], in_=eidx[:, x0:x0 + IXC])
                    gt = gp.tile([CHE, GCH * TROW], BF16)
                    gv = gt[:].rearrange("p (c e) -> p c e", e=TROW)
                    for b in range(NB):
                        nc.gpsimd.dma_gather(
                            gv[:, b * GS * CB:(b + 1) * GS * CB, :],
                            T2b[b][:, :],
                            ix[:, b * 64:(b + 1) * 64],
                            GS * CB * CHE, GS * CB * CHE, TROW,
                            queue_num=(5 * g + b) % 4)

                    ers = ep.tile([CHE, GS * 128], BF16)
                    nc.gpsimd.dma_gather(
                        ers[:].rearrange("p (c e) -> p c e", e=128),
                        erloc[:, :], ix[:, NB * 64:IXC],
                        GS * SN, GS * SN, 128, queue_num=(5 * g + 4) % 4)
                    erv = ers[:].rearrange("p (c e) -> p c e", e=128)

                    # er expansion: slot -> edge via host-built Pc one-hots
                    Pc = pcp.tile([SN, GED], FP8)
                    nc.scalar.dma_start(
                        out=Pc[:], in_=pcd[:, g * GED:(g + 1) * GED])
                    erx_ps = erpsp.tile([CHE, GCH * H], F32)
                    for c in range(GCH):
                        s = (c % (GS * CB)) // CB
                        nc.tensor.matmul(
                            out=erx_ps[:, c * H:(c + 1) * H],
                            lhsT=Pc[:, c * CHE:(c + 1) * CHE],
                            rhs=erv[:, s, 0:H],
                            start=True, stop=True,
                        )
                    erx = ep.tile([CHE, GCH * H], BF16, tag="erx")
                    nc.scalar.activation(out=erx[:], in_=erx_ps[:],
                                         func=ACT.Copy)

                    u = uxp.tile([CHE, GCH * H], F32, tag="u")
                    nc.vector.tensor_tensor(
                        out=u[:].rearrange("p (c h) -> p c h", h=H),
                        in0=gv[:, :, HD:HD + H],
                        in1=erx[:].rearrange("p (c h) -> p c h", h=H),
                        op=AOT.add,
                    )
                    u2 = uxp.tile([CHE, GCH * H], F32, tag="u2")
                    nc.vector.scalar_tensor_tensor(
                        out=u2[:], in0=u[:], scalar=NEG, in1=u[:],
                        op0=AOT.mult, op1=AOT.max)
                    ex = uxp.tile([CHE, GCH * H], BF16, tag="ex")
                    nc.scalar.activation(out=ex[:], in_=u2[:], func=ACT.Exp)
                    exv = ex[:].rearrange("p (c h) -> p c h", h=H)

                    P_t = ppool.tile([CHE, GCH * SN], FP8)
                    nc.scalar.dma_start(
                        out=P_t[:],
                        in_=ptd[:, g * GCH * SN:(g + 1) * GCH * SN])

                    mx = mxp.tile([CHE, GCH * ML], BF16)
                    mv = mx[:].rearrange("p (c w) -> p c w", w=ML)
                    nc.scalar.activation(out=mv[:, :, HD:HD + H], in_=exv,
                                         func=ACT.Copy)
                    nc.vector.tensor_tensor(
                        out=mv[:, :, 0:HD].rearrange(
                            "p c (h d) -> p c h d", h=H),
                        in0=gv[:, :, 0:HD].rearrange(
                            "p c (h d) -> p c h d", h=H),
                        in1=exv[:, :, :, None].broadcast_to([CHE, GCH, H, D]),
                        op=AOT.mult,
                    )

                    for s in range(GS):
                        ps = p2ps.tile([SN, ML], F32)
                        for b in range(NB):
                            for i in range(CB):
                                c = b * GS * CB + s * CB + i
                                nc.tensor.matmul(
                                    out=ps[:],
                                    lhsT=P_t[:, c * SN:(c + 1) * SN],
                                    rhs=mx[:, c * ML:(c + 1) * ML],
                                    start=(b == 0 and i == 0),
                                    stop=(b == NB - 1 and i == CB - 1),
                                )
                        r0 = rp.tile([SN, H], F32, tag="r0")
                        nc.vector.tensor_scalar_max(r0[:], ps[:, HD:HD + H],
                                                    1e-30)
                        r1 = rp.tile([SN, H], F32, tag="r1")
                        nc.vector.reciprocal(r1[:], r0[:])
                        stg = sop.tile([SN, HD], F32)
                        nc.vector.tensor_tensor(
                            out=stg[:].rearrange("p (h d) -> p h d", h=H),
                            in0=ps[:, 0:HD].rearrange("p (h d) -> p h d", h=H),
                            in1=r1[:, :, None].broadcast_to([SN, H, D]),
                            op=AOT.mult,
                        )
                        nc.sync.dma_start(
                            out=out[(g * GS + s) * SN:(g * GS + s + 1) * SN, :],
                            in_=stg[:])

    nc.compile()
    return nc


_NC_CACHE = {}
LAST_RESULTS = None


def _get_program(ngroups):
    if ngroups not in _NC_CACHE:
        _NC_CACHE[ngroups] = _build(ngroups)
    return _NC_CACHE[ngroups]


def kernel(feat, fc_w, attn_l, attn_r, src, dst):
    import ml_dtypes
    from concourse.bass_utils import run_bass_kernel_spmd

    feat = np.asarray(feat, dtype=np.float32)
    fc_w = np.ascontiguousarray(np.asarray(fc_w, dtype=np.float32))
    attn_l = np.asarray(attn_l, dtype=np.float32)
    attn_r = np.asarray(attn_r, dtype=np.float32)
    src = np.asarray(src).astype(np.int64)
    dst = np.asarray(dst).astype(np.int64)

    edatas, node_maps, ngroups = _pack(src, dst)

    featp = np.zeros((NPAD, F), np.float32)
    featp[:N] = feat
    featLT_bf = np.ascontiguousarray(featp.T).astype(ml_dtypes.bfloat16)
    featT_bf = featLT_bf
    attn = np.concatenate(
        [attn_l.reshape(-1), attn_r.reshape(-1)]).reshape(1, 2 * HD)
    attn = np.ascontiguousarray(attn.astype(np.float32))

    nc = _get_program(ngroups)
    in_maps = [
        {"featT": featT_bf,
         "featL": np.ascontiguousarray(
             featLT_bf[:, k * LOCN:(k + 1) * LOCN]),
         "fcw": fc_w, "attn": attn,
         "eidx": edatas[k]["eidx"], "ptd": edatas[k]["ptd"],
         "pcd": edatas[k]["pcd"]}
        for k in range(NCORES)
    ]
    res = run_bass_kernel_spmd(nc, in_maps, core_ids=list(range(NCORES)))
    global LAST_RESULTS
    LAST_RESULTS = res

    outf = np.zeros((N, HD), np.float32)
    for k in range(NCORES):
        o = np.asarray(res.results[k]["out"])
        nm = node_maps[k]
        m = nm >= 0
        outf[nm[m]] = o[m]
    return outf
